# revision 26
# baseline (speedup 1.0000x reference)
"""LISTA unrolled encoder (nn_AE_63548336111686) on 8 TRN2 NeuronCores.

reference:
    A   = I - 0.1 * W^T W                      [P, P]
    WTx = 0.1 * W^T x                          per batch col
    z_{k+1} = softthresh(A z_k + WTx, 0.01)    30 layers, z_0 = 0
    xhat = W z_30
    returns (xhat, z_30)

Strategy: pure data-parallel over batch (4096 -> 8 cores x 512), one pass
per core, features on partitions. The iteration is FACTORED through W
instead of materializing A:

    pre = A z + WTx = z + 0.1 * W^T (x - W z)

which is 2*M*P MACs per column instead of P*P -- a 2x FLOP reduction --
and needs no [2048 x 2048] operand at all. Per layer:
  - u-phase:  u[m] = sum_q Wdec[q, m*128:+128]^T @ z[q]   (64 matmuls)
              v[m] = f16(x[m] - u[m])                      (DVE, PSUM read)
  - d-phase:  d[p] = sum_m (0.1 W)[m, p*128:+128]^T @ v[m] (64 matmuls)
              pre  = d + z_prev[p]                         (DVE, PSUM read)
              z'   = relu(pre-l) - relu(-pre-l)            (2x ACT + DVE)
Matmuls run in fp16 (1 cycle/row + fast weight load; fp32 PSUM
accumulate; z/v stored fp16) -- measured end-to-end rel err ~9e-4
(gate 2e-2). Layer 1 is just the d-phase with v = x (z_0 = 0); decode
is a u-phase with fp32 eviction. No cross-core communication.

The soft-threshold eviction is a single fused custom DVE op
(z' = r - clip(r, -l, +l), r = psum + z). Keeping ScalarE idle matters
beyond op count: with the 2x-ScalarE-Relu eviction variant the chip
downclocks the PE 2.4 -> 2.0 GHz under the sustained combined load
(measured 259 ns vs 216 ns per [128x128]x[128x512] matmul).

Measured on 8xTRN2 (neuron-profile): HW exec ~852 us, PE busy 97%,
3840 matmuls/core at 216 ns issue rate (98.5% of the fp16 stream
bound), zero steady-state PE gaps.
"""

import os
import sys

import numpy as np

for _p in ("/opt/trn_rl_repo",):
    if _p not in sys.path:
        sys.path.insert(0, _p)

import concourse.bacc as bacc
import concourse.mybir as mybir
import concourse.tile as tile
from concourse import dve_ops
from concourse.bass_utils import run_bass_kernel_spmd
from concourse.dve_spec import C0, C1, Spec, Src0, Src1, lower, maxx, minn
from concourse.dve_table_gen import dve_ver_for
from concourse.dve_uop import DveOpSpec


def _register_softshrink():
    """Custom fused DVE op: out = r - clip(r, s0, s1), r = in0 + in1.

    With s0=-lam, s1=+lam this is softthresh(psum + z_old) in ONE Vector-op
    (replaces DVE add + 2x ScalarE relu + DVE sub). Registered at runtime;
    the uops sha is computed on the spot (nothing to pin against)."""
    name = "SOFTSHRINK_ADD_ANT"
    if any(op.name == name for op in dve_ops.OPS):
        return next(op for op in dve_ops.OPS if op.name == name)
    r = Src0 + Src1
    spec = Spec(
        body=r - minn(maxx(r, C0), C1),
        reference=lambda in0, in1, s0, s1, imm2=0.0: (
            (in0 + in1) - np.clip(in0 + in1, s0, s1)),
    )
    shas = {}
    for ver in ("v3", "v4"):
        try:
            shas[ver] = DveOpSpec(name=name, uops=lower(spec, ver=ver),
                                  rd1_en=True).sha(ver)
        except Exception:
            pass
    op = dve_ops.DveOp(name, spec, subdim=False, uops_sha=shas)
    dve_ops.OPS.append(op)
    dve_ops.CUSTOM_DVE_SPECS[name] = spec
    dve_ops._SUB_OPCODE_FOR_NAME[name] = (
        dve_ops._CUSTOM_DVE_ROW_BASE + len(dve_ops.OPS) - 1)
    assert max(dve_ops._SUB_OPCODE_FOR_NAME.values()) < 0x20
    return op


SOFTSHRINK_ADD = _register_softshrink()

# Problem constants (hardcoded; must match reference.py)
M, P, B = 512, 2048, 4096
NUM_LAYERS = 30
LAM = 0.1
STEP = 0.1
THRESH = LAM * STEP  # 0.01

NCORES = 8
BS = B // NCORES  # 512 batch columns per core
PT = P // 128     # 16 p tiles
MC = M // 128     # 4 m chunks

F32 = mybir.dt.float32

# Matmul-path dtype: fp16 default, fp32r fallback via env.
if os.environ.get("LISTA_DTYPE", "f16") == "f32r":
    DT, NPDT = mybir.dt.float32r, np.float32
else:
    DT, NPDT = mybir.dt.float16, np.float16

# stash of the last BassKernelResults (for test.py profiling)
LAST_RESULTS = None


def _softthresh(nc, zpool, psum, z_old):
    """z_new = softthresh(psum + z_old, THRESH) -> fp16 tile, one DVE op."""
    z_p = zpool.tile([128, BS], DT, name="z", tag="z")
    nc.vector._custom_dve(SOFTSHRINK_ADD, out=z_p, in0=psum, in1=z_old,
                          s0=-THRESH, s1=THRESH)
    return z_p


def build_nc():
    nc = bacc.Bacc("TRN2", target_bir_lowering=False)
    x_t = nc.declare_dram_parameter("x_t", [M, BS], DT, isOutput=False)
    w_enc = nc.declare_dram_parameter("w_enc", [M, P], DT, isOutput=False)
    w_dec = nc.declare_dram_parameter("w_dec", [P, M], DT, isOutput=False)
    z_out = nc.declare_dram_parameter("z_out", [P, BS], DT, isOutput=True)
    xh_out = nc.declare_dram_parameter("xh_out", [M, BS], F32, isOutput=True)

    with tile.TileContext(nc) as tc, \
         tc.tile_pool(name="wpool", bufs=1) as wpool, \
         tc.tile_pool(name="zpool", bufs=32) as zpool, \
         tc.tile_pool(name="vpool", bufs=8) as vpool, \
         tc.tile_pool(name="evict", bufs=3) as evict, \
         tc.tile_pool(name="ps", bufs=8, space="PSUM") as ps:

        z_zero = wpool.tile([128, BS], DT, name="z_zero", tag="z_zero")
        nc.vector.memset(z_zero, 0.0)

        # --- resident weights + x (all fp16: ~40 KB/partition total)
        # we loaded in 512-col chunks; issue order feeds layer-1 p=0 first:
        # xt[m] + we[m] chunk 0, interleaved, then the later we chunks.
        xt_sb = [wpool.tile([128, BS], DT, name=f"xt_{m}", tag=f"xt_{m}")
                 for m in range(MC)]
        we_sb = [wpool.tile([128, P], DT, name=f"we_{m}", tag=f"we_{m}")
                 for m in range(MC)]
        for m in range(MC):
            nc.sync.dma_start(out=xt_sb[m], in_=x_t[m * 128:(m + 1) * 128, :])
            nc.sync.dma_start(out=we_sb[m][:, 0:512],
                              in_=w_enc[m * 128:(m + 1) * 128, 0:512])
        for c in range(1, 4):
            for m in range(MC):
                nc.sync.dma_start(
                    out=we_sb[m][:, c * 512:(c + 1) * 512],
                    in_=w_enc[m * 128:(m + 1) * 128, c * 512:(c + 1) * 512])

        # PE warm-up: dummy matmuls on the resident zero tile while the
        # input DMAs land, so the HAM un-throttles (1.2 -> 2.4 GHz) before
        # the real stream starts.
        warm_ps = ps.tile([128, BS], F32, name="warm_ps", tag="ps")
        for k in range(24):
            nc.tensor.matmul(warm_ps, z_zero[:, 0:128], z_zero,
                             start=(k == 0), stop=(k == 23))

        def d_phase(vs, z_old):
            """z_new[p] = softthresh(sum_m we[m][:,p]^T @ vs[m] + z_old[p])"""
            z_new = []
            for p in range(PT):
                psum = ps.tile([128, BS], F32, name="ps_d", tag="ps")
                for m in range(MC):
                    nc.tensor.matmul(psum,
                                     we_sb[m][:, p * 128:(p + 1) * 128],
                                     vs[m],
                                     start=(m == 0), stop=(m == MC - 1))
                z_new.append(_softthresh(nc, zpool, psum,
                                         z_old[p] if z_old else z_zero))
            return z_new

        # --- layer 1: v = x  (u = W z_0 = 0)
        z_prev = d_phase(xt_sb, None)

        # Wdec loads issued after layer 1 so they don't compete with the
        # layer-1-critical we/xt DMAs; needed from iteration 2 (~27us in).
        wd_sb = []
        for q in range(PT):
            wd_q = wpool.tile([128, M], DT, name=f"wd_{q}", tag=f"wd_{q}")
            nc.sync.dma_start(out=wd_q, in_=w_dec[q * 128:(q + 1) * 128, :])
            wd_sb.append(wd_q)

        # --- layers 2..30
        for _it in range(1, NUM_LAYERS):
            vs = []
            for m in range(MC):
                psum = ps.tile([128, BS], F32, name="ps_u", tag="ps")
                for q in range(PT):
                    nc.tensor.matmul(psum,
                                     wd_sb[q][:, m * 128:(m + 1) * 128],
                                     z_prev[q],
                                     start=(q == 0), stop=(q == PT - 1))
                v_m = vpool.tile([128, BS], DT, name="v", tag="v")
                nc.vector.tensor_sub(v_m, xt_sb[m], psum)  # v = x - W z
                vs.append(v_m)
            z_prev = d_phase(vs, z_prev)

        # --- z output
        for p in range(PT):
            nc.sync.dma_start(out=z_out[p * 128:(p + 1) * 128, :],
                              in_=z_prev[p])

        # --- decode: xhat[m] = sum_q wd[q][:, m]^T @ z[q]  (u-phase, f32 out)
        for m in range(MC):
            psum = ps.tile([128, BS], F32, name="ps_dec", tag="ps")
            for q in range(PT):
                nc.tensor.matmul(psum,
                                 wd_sb[q][:, m * 128:(m + 1) * 128],
                                 z_prev[q],
                                 start=(q == 0), stop=(q == PT - 1))
            xh_m = evict.tile([128, BS], F32, name="xh", tag="xh")
            for c in range(2):  # halves so the output DMA starts earlier
                cs = slice(c * (BS // 2), (c + 1) * (BS // 2))
                nc.vector.tensor_copy(xh_m[:, cs], psum[:, cs])
                nc.sync.dma_start(out=xh_out[m * 128:(m + 1) * 128, cs],
                                  in_=xh_m[:, cs])
    nc.compile()
    return nc


def make_in_maps(x, W):
    x = np.asarray(x, dtype=np.float32).reshape(B, M)
    W = np.asarray(W, dtype=np.float32)
    wenc = np.ascontiguousarray(np.float32(STEP) * W).astype(NPDT)
    wdec = np.ascontiguousarray(W.T).astype(NPDT)
    in_maps = []
    for c in range(NCORES):
        xs = np.ascontiguousarray(x[c * BS:(c + 1) * BS].T)  # [M, BS]
        in_maps.append({"x_t": xs.astype(NPDT), "w_enc": wenc,
                        "w_dec": wdec})
    return in_maps


def kernel(x, W):
    global LAST_RESULTS
    nc = build_nc()
    in_maps = make_in_maps(x, W)
    trace = bool(int(os.environ.get("LISTA_TRACE", "0")))
    res = run_bass_kernel_spmd(nc, in_maps, core_ids=list(range(NCORES)),
                               trace=trace)
    LAST_RESULTS = res
    zs, xhs = [], []
    for c in range(NCORES):
        zs.append(np.asarray(res.results[c]["z_out"], np.float32).T)
        xhs.append(np.asarray(res.results[c]["xh_out"], np.float32).T)
    zT = np.concatenate(zs, axis=0)[..., None].astype(np.float32)
    xhat = np.concatenate(xhs, axis=0)[..., None].astype(np.float32)
    return (xhat, zT)


# revision 28
# speedup vs baseline: 1.1980x; 1.1980x over previous
"""LISTA unrolled encoder (nn_AE_63548336111686) on 8 TRN2 NeuronCores.

reference:
    A   = I - 0.1 * W^T W                      [P, P]
    WTx = 0.1 * W^T x                          per batch col
    z_{k+1} = softthresh(A z_k + WTx, 0.01)    30 layers, z_0 = 0
    xhat = W z_30
    returns (xhat, z_30)

Strategy: pure data-parallel over batch (4096 -> 8 cores x 512), one pass
per core, features on partitions. The iteration is FACTORED through W
instead of materializing A:

    pre = A z + WTx = z + 0.1 * W^T (x - W z)

which is 2*M*P MACs per column instead of P*P -- a 2x FLOP reduction --
and needs no [2048 x 2048] operand at all. Per layer:
  - u-phase:  u[m] = sum_q Wdec[q, m*128:+128]^T @ z[q]   (64 matmuls)
              v[m] = f16(x[m] - u[m])                      (DVE, PSUM read)
  - d-phase:  d[p] = sum_m (0.1 W)[m, p*128:+128]^T @ v[m] (64 matmuls)
              pre  = d + z_prev[p]                         (DVE, PSUM read)
              z'   = relu(pre-l) - relu(-pre-l)            (2x ACT + DVE)
Matmuls run in fp16 (1 cycle/row + fast weight load; fp32 PSUM
accumulate; z/v stored fp16) -- measured end-to-end rel err ~9e-4
(gate 2e-2). Layer 1 is just the d-phase with v = x (z_0 = 0); decode
is a u-phase with fp32 eviction. No cross-core communication.

The soft-threshold eviction is a single fused custom DVE op
(z' = r - clip(r, -l, +l), r = psum + z). Keeping ScalarE idle matters
beyond op count: with the 2x-ScalarE-Relu eviction variant the chip
downclocks the PE 2.4 -> 2.0 GHz under the sustained combined load
(measured 259 ns vs 216 ns per [128x128]x[128x512] matmul).

Measured on 8xTRN2 (neuron-profile): HW exec ~852 us, PE busy 97%,
3840 matmuls/core at 216 ns issue rate (98.5% of the fp16 stream
bound), zero steady-state PE gaps.
"""

import os
import sys

import numpy as np

for _p in ("/opt/trn_rl_repo",):
    if _p not in sys.path:
        sys.path.insert(0, _p)

import concourse.bacc as bacc
import concourse.mybir as mybir
import concourse.tile as tile
from concourse import dve_ops
from concourse.bass_utils import run_bass_kernel_spmd
from concourse.dve_spec import C0, C1, Spec, Src0, Src1, lower, maxx, minn
from concourse.dve_table_gen import dve_ver_for
from concourse.dve_uop import DveOpSpec


def _register_softshrink():
    """Custom fused DVE op: out = r - clip(r, s0, s1), r = in0 + in1.

    With s0=-lam, s1=+lam this is softthresh(psum + z_old) in ONE Vector-op
    (replaces DVE add + 2x ScalarE relu + DVE sub). Registered at runtime;
    the uops sha is computed on the spot (nothing to pin against)."""
    name = "SOFTSHRINK_ADD_ANT"
    if any(op.name == name for op in dve_ops.OPS):
        return next(op for op in dve_ops.OPS if op.name == name)
    r = Src0 + Src1
    spec = Spec(
        body=r - minn(maxx(r, C0), C1),
        reference=lambda in0, in1, s0, s1, imm2=0.0: (
            (in0 + in1) - np.clip(in0 + in1, s0, s1)),
    )
    shas = {}
    for ver in ("v3", "v4"):
        try:
            shas[ver] = DveOpSpec(name=name, uops=lower(spec, ver=ver),
                                  rd1_en=True).sha(ver)
        except Exception:
            pass
    op = dve_ops.DveOp(name, spec, subdim=False, uops_sha=shas)
    dve_ops.OPS.append(op)
    dve_ops.CUSTOM_DVE_SPECS[name] = spec
    dve_ops._SUB_OPCODE_FOR_NAME[name] = (
        dve_ops._CUSTOM_DVE_ROW_BASE + len(dve_ops.OPS) - 1)
    assert max(dve_ops._SUB_OPCODE_FOR_NAME.values()) < 0x20
    return op


SOFTSHRINK_ADD = _register_softshrink()

# Problem constants (hardcoded; must match reference.py)
M, P, B = 512, 2048, 4096
NUM_LAYERS = 30
LAM = 0.1
STEP = 0.1
THRESH = LAM * STEP  # 0.01

NCORES = 8
BS = B // NCORES  # 512 batch columns per core
PT = P // 128     # 16 p tiles
MC = M // 128     # 4 m chunks

F32 = mybir.dt.float32

# Matmul-path dtype: fp16 default, fp32r fallback via env.
if os.environ.get("LISTA_DTYPE", "f16") == "f32r":
    DT, NPDT = mybir.dt.float32r, np.float32
else:
    DT, NPDT = mybir.dt.float16, np.float16

# stash of the last BassKernelResults (for test.py profiling)
LAST_RESULTS = None


def _softthresh(nc, zpool, psum, z_old):
    """z_new = softthresh(psum + z_old, THRESH) -> fp16 tile, one DVE op."""
    z_p = zpool.tile([128, BS], DT, name="z", tag="z")
    nc.vector._custom_dve(SOFTSHRINK_ADD, out=z_p, in0=psum, in1=z_old,
                          s0=-THRESH, s1=THRESH)
    return z_p


def build_nc():
    nc = bacc.Bacc("TRN2", target_bir_lowering=False)
    x_t = nc.declare_dram_parameter("x_t", [M, BS], DT, isOutput=False)
    w_enc = nc.declare_dram_parameter("w_enc", [M, P], DT, isOutput=False)
    w_dec = nc.declare_dram_parameter("w_dec", [P, M], DT, isOutput=False)
    z_out = nc.declare_dram_parameter("z_out", [P, BS], DT, isOutput=True)
    xh_out = nc.declare_dram_parameter("xh_out", [M, BS], F32, isOutput=True)

    with tile.TileContext(nc) as tc, \
         tc.tile_pool(name="wpool", bufs=1) as wpool, \
         tc.tile_pool(name="zpool", bufs=32) as zpool, \
         tc.tile_pool(name="vpool", bufs=8) as vpool, \
         tc.tile_pool(name="evict", bufs=3) as evict, \
         tc.tile_pool(name="ps", bufs=8, space="PSUM") as ps:

        z_zero = wpool.tile([128, BS], DT, name="z_zero", tag="z_zero")
        nc.vector.memset(z_zero, 0.0)

        # --- resident weights + x (all fp16: ~40 KB/partition total)
        # we loaded in 512-col chunks; issue order feeds layer-1 p=0 first:
        # xt[m] + we[m] chunk 0, interleaved, then the later we chunks.
        xt_sb = [wpool.tile([128, BS], DT, name=f"xt_{m}", tag=f"xt_{m}")
                 for m in range(MC)]
        we_sb = [wpool.tile([128, P], DT, name=f"we_{m}", tag=f"we_{m}")
                 for m in range(MC)]
        for m in range(MC):
            nc.sync.dma_start(out=xt_sb[m], in_=x_t[m * 128:(m + 1) * 128, :])
            nc.sync.dma_start(out=we_sb[m][:, 0:512],
                              in_=w_enc[m * 128:(m + 1) * 128, 0:512])
        for c in range(1, 4):
            for m in range(MC):
                nc.sync.dma_start(
                    out=we_sb[m][:, c * 512:(c + 1) * 512],
                    in_=w_enc[m * 128:(m + 1) * 128, c * 512:(c + 1) * 512])

        # PE warm-up: dummy matmuls on the resident zero tile while the
        # input DMAs land, so the HAM un-throttles (1.2 -> 2.4 GHz) before
        # the real stream starts.
        warm_ps = ps.tile([128, BS], F32, name="warm_ps", tag="ps")
        for k in range(14):
            nc.tensor.matmul(warm_ps, z_zero[:, 0:128], z_zero,
                             start=(k == 0), stop=(k == 13))

        def d_phase(vs, z_old):
            """z_new[p] = softthresh(sum_m we[m][:,p]^T @ vs[m] + z_old[p])"""
            z_new = []
            for p in range(PT):
                psum = ps.tile([128, BS], F32, name="ps_d", tag="ps")
                for m in range(MC):
                    nc.tensor.matmul(psum,
                                     we_sb[m][:, p * 128:(p + 1) * 128],
                                     vs[m],
                                     start=(m == 0), stop=(m == MC - 1))
                z_new.append(_softthresh(nc, zpool, psum,
                                         z_old[p] if z_old else z_zero))
            return z_new

        # --- layer 1: v = x  (u = W z_0 = 0)
        z_prev = d_phase(xt_sb, None)

        # Wdec loads issued after layer 1 so they don't compete with the
        # layer-1-critical we/xt DMAs; needed from iteration 2 (~27us in).
        wd_sb = []
        for q in range(PT):
            wd_q = wpool.tile([128, M], DT, name=f"wd_{q}", tag=f"wd_{q}")
            nc.sync.dma_start(out=wd_q, in_=w_dec[q * 128:(q + 1) * 128, :])
            wd_sb.append(wd_q)

        # --- layers 2..30
        for _it in range(1, NUM_LAYERS):
            vs = []
            for m in range(MC):
                psum = ps.tile([128, BS], F32, name="ps_u", tag="ps")
                for q in range(PT):
                    nc.tensor.matmul(psum,
                                     wd_sb[q][:, m * 128:(m + 1) * 128],
                                     z_prev[q],
                                     start=(q == 0), stop=(q == PT - 1))
                v_m = vpool.tile([128, BS], DT, name="v", tag="v")
                nc.vector.tensor_sub(v_m, xt_sb[m], psum)  # v = x - W z
                vs.append(v_m)
            z_prev = d_phase(vs, z_prev)

        # --- z output
        for p in range(PT):
            nc.sync.dma_start(out=z_out[p * 128:(p + 1) * 128, :],
                              in_=z_prev[p])

        # --- decode: xhat[m] = sum_q wd[q][:, m]^T @ z[q]  (u-phase, f32 out)
        for m in range(MC):
            psum = ps.tile([128, BS], F32, name="ps_dec", tag="ps")
            for q in range(PT):
                nc.tensor.matmul(psum,
                                 wd_sb[q][:, m * 128:(m + 1) * 128],
                                 z_prev[q],
                                 start=(q == 0), stop=(q == PT - 1))
            xh_m = evict.tile([128, BS], F32, name="xh", tag="xh")
            for c in range(4):  # quarters so the output DMA starts earlier
                cs = slice(c * (BS // 4), (c + 1) * (BS // 4))
                nc.vector.tensor_copy(xh_m[:, cs], psum[:, cs])
                nc.sync.dma_start(out=xh_out[m * 128:(m + 1) * 128, cs],
                                  in_=xh_m[:, cs])
    nc.compile()
    return nc


def make_in_maps(x, W):
    x = np.asarray(x, dtype=np.float32).reshape(B, M)
    W = np.asarray(W, dtype=np.float32)
    wenc = np.ascontiguousarray(np.float32(STEP) * W).astype(NPDT)
    wdec = np.ascontiguousarray(W.T).astype(NPDT)
    in_maps = []
    for c in range(NCORES):
        xs = np.ascontiguousarray(x[c * BS:(c + 1) * BS].T)  # [M, BS]
        in_maps.append({"x_t": xs.astype(NPDT), "w_enc": wenc,
                        "w_dec": wdec})
    return in_maps


def kernel(x, W):
    global LAST_RESULTS
    nc = build_nc()
    in_maps = make_in_maps(x, W)
    trace = bool(int(os.environ.get("LISTA_TRACE", "0")))
    res = run_bass_kernel_spmd(nc, in_maps, core_ids=list(range(NCORES)),
                               trace=trace)
    LAST_RESULTS = res
    zs, xhs = [], []
    for c in range(NCORES):
        zs.append(np.asarray(res.results[c]["z_out"], np.float32).T)
        xhs.append(np.asarray(res.results[c]["xh_out"], np.float32).T)
    zT = np.concatenate(zs, axis=0)[..., None].astype(np.float32)
    xhat = np.concatenate(xhs, axis=0)[..., None].astype(np.float32)
    return (xhat, zT)
